# revision 42
# baseline (speedup 1.0000x reference)
"""Trainium2 Bass kernel for a 2-layer message-passing GNN (BaselineGNN).

Reference computation (N=4096 nodes, IN=512, HID=4096, E=65536 edges):
    h   = x @ We.T + be                                   [N, HID]
    for W, b in ((W1, b1), (W2, b2)):
        aggr = segment_sum(h[col], row)                   [N, HID]
        h    = relu(aggr @ W.T + b)
    hm  = mean(h, axis=1)                                 [N]
    z   = relu(hm @ Wc1.T + bc1)                          [HID//2]
    out = (z @ Wc2.T + bc2).squeeze(-1)                   scalar

Strategy (8 NeuronCores, node-parallel, collective-hiding reorder):
  * segment_sum == A @ h with A the [N, N] adjacency-count matrix (0.4%
    dense).  A's entries are small integer counts -> exactly representable
    in fp8-e4m3, so aggregation runs as a dense TensorEngine matmul.
  * Nodes are sharded: core c owns rows 512c..512c+512.
  * Layer 1 is low-rank through the embed bottleneck:
        h1_c = relu((A_c @ x_ext) @ (We_ext.T @ W1.T) + b1)
    with the weight product folded on the host (x_ext carries a ones
    column, We_ext.T a b_embed row).
  * Layer 2 is REORDERED vs the naive form to hide the collective:
        h2 = relu(A @ h1 @ W2.T + b2) = relu(A @ (h1 @ W2.T) + b2)
    Each core computes p_c = h1_c @ W2.T *locally* (W2 replicated) BEFORE
    any cross-core exchange -- ~150us of PE work that overlaps the
    first-collective rendezvous/launch-skew barrier AND the p AllGather.
    p is AllGathered in column halves; the final aggregation
    aggr2 = A_c @ p_full then runs per half with relu+b2+row-sum fused
    into the PSUM eviction (ScalarE accum_out) -> hm directly.  No
    second weight matmul after the collective, so the PE pipeline is
    near-continuous: M1 -> M3 -> M5 (local, hides AG) -> M4 -> head.
  * Pipeline per core: M1 tT=(A_c@x_ext).T [fp8 DR, split into source
    halves so it starts before the operand caches fully land] -> M3 h1T
    [bf16, feature-major so h1T is the kxm cache for M5] -> M5 halves
    p_c = h1T.T @ (W2.T*16) [fp8 DR] -> AllGather half (fp8) ->
    M4 halves aggr2 = at8.T @ p_full [fp8 DR] with relu+row-sum fused
    eviction -> per-half partial z = Wc1@hm_half AllReduced early so only
    the last AR sits on the tail -> epilogue.
  * Chained matmuls hand intermediates over in SBUF caches (tT_c, h1T_c,
    at8_c) via mxn_subtile_producer / kxm/kxn_cache.
  * fp8 operands that are not exact (x, h1, p, W2) rely on fp32 PSUM
    accumulation and the mean-pool/classifier averaging to wash the 6%
    element-level rounding down to ~4e-4 relative error on the scalar
    output; W2 is pre-scaled by 16 to sit in e4m3's normal range (the
    hm normalization divides it back out).  The classifier runs bf16/fp32.
  * A dummy AllReduce issued first absorbs the multi-core launch skew on
    the collective engine while the PE computes M1/M3/M5.

Everything is expressed "transposed-free": every matmul is
mxn = kxm.T @ kxn with operands stored so no transposes are ever needed.
"""

import contextlib

import numpy as np
import ml_dtypes

import concourse.bass as bass
import concourse.mybir as mybir
import concourse.tile as tile
from concourse import bacc
from concourse.bass_interp import get_hw_module
from concourse.bass_utils import run_bass_kernel_spmd
from concourse.kernels.tile_matmul import (
    composable_matmul_tile_kernel,
    dma_from_dram_kxm,
    dma_from_dram_kxn,
    dma_to_dram_mxn,
    k_pool_min_bufs,
    scalar_copyback,
    scalar_scale,
)

N = 4096          # nodes
IN_DIM = 512
HID = 4096
NCORES = 8
S = N // NCORES           # nodes per core (512)
KE = 640                  # extended embed contraction (512 + 1 ones col, padded to 5*128)
CHID = HID // 2           # classifier hidden (2048)

BF16 = mybir.dt.bfloat16
F32 = mybir.dt.float32
FP8 = mybir.dt.float8e4

USE_FP8 = True
# p = h1 @ (W2.T * WSCALE) is materialized in fp8-e4m3 (range +-448):
# p_true has absmax ~12, so WSCALE=16 centers it (~200 max) with margin.
WSCALE = 16.0
# layer-1 path in fp8 DoubleRow: wcb is scaled by WCB_SCALE (its entries
# are ~9e-3, below e4m3's normal range), h1T carries the factor (absmax
# 15.6*16=250 < 448; x32 would clip), and the p eviction divides it back
# out so p stays WSCALE-scaled.
M3_FP8 = False
WCB_SCALE = 16.0
KEP = 768 if M3_FP8 else KE   # KE padded to a DoubleRow-even k (6*128)

NAG = 2                   # AllGather column-chunk count
HQ = HID // NAG

_COMPILED = {}


def _m4_reducer(nc_b, bias_sb, accum, n_off):
    """PSUM->SBUF evict for the final aggregation:
    sbuf = relu(psum + b2[n_slice]); row-sums stream into accum.

    bias_sb: [128, HID] SBUF tile (b2 broadcast across partitions), or None
    when b2 is all-zero.  accum: [128, MSUB, NT] fp32; the relu
    row-sum-accumulates into accum[:, m_subtile, n_off + n_tile].
    """

    def _reducer(nc, psum, sbuf, md):
        src_ap = psum[:, : md.n_slice_size]
        ni = n_off + md.n_tile_idx * md.n_subtiles + md.n_subtile_idx
        if bias_sb is not None:
            start = (
                n_off * md.n_tile
                + md.n_tile_idx * md.n_tile
                + md.n_subtile_idx * md.n_subtile
            )
            nc.vector.tensor_add(
                out=sbuf[:, :, : md.n_slice_size],
                in0=src_ap,
                in1=bias_sb[:, start : start + md.n_slice_size],
            )
            src_ap = sbuf
        nc.scalar.activation(
            out=sbuf,
            in_=src_ap,
            func=mybir.ActivationFunctionType.Relu,
            accum_out=accum[:, md.m_subtile_idx, ni : ni + 1],
        )

    return _reducer


def _m1_combine_reducer(tT_h, tT_c):
    """Second M1 half eviction: tT_c = fp(psum + tT_h) (combine k-halves)."""

    def _reducer(nc, psum, sbuf, md):
        mi = md.m_subtiles * md.m_tile_idx + md.m_subtile_idx
        nc.vector.tensor_add(
            out=tT_c[:, mi : mi + 1, : md.n_slice_size],
            in0=psum[:, : md.n_slice_size],
            in1=tT_h[:, mi : mi + 1, : md.n_slice_size],
        )

    return _reducer


def _m3_reducer(nc_b, b1c_sb):
    """PSUM->SBUF evict for h1T: sbuf = relu(psum + b1[m_slice]).

    h1T is feature-major, so the layer-1 bias is per-partition:
    b1c_sb is [128, HID//128] with b1c[p, m] = b1[m*128 + p], or None
    when b1 is all-zero.
    """

    def _reducer(nc, psum, sbuf, md):
        if b1c_sb is None:
            nc.scalar.activation(
                out=sbuf, in_=psum, func=mybir.ActivationFunctionType.Relu
            )
        else:
            mi = md.m_tile_idx * md.m_subtiles + md.m_subtile_idx
            nc.vector.tensor_add(
                out=sbuf[:, :, : md.n_slice_size],
                in0=psum[:, : md.n_slice_size],
                in1=b1c_sb[:, mi : mi + 1].to_broadcast(
                    [128, 1, md.n_slice_size]
                ),
            )
            nc.vector.tensor_scalar_max(sbuf, sbuf, 0.0)

    return _reducer


def _matmul_custom(ctx, tc, kxm_ap, kxn_ap, reducer, consumer, output_type=F32,
                   psum_n_bufs=2, kxm_cache=None, kxn_cache=None, producer=None,
                   max_k_tile=512, stream_ap=None, kxn_pool=None,
                   reducer_override=None):
    """composable matmul with custom psum reducer / mxn consumer / SBUF caches.

    kxm_cache / kxn_cache: [128, K//128, M-or-N] SBUF tiles already holding
    the operand (no DMA is issued for that side).  producer: optional
    mxn_subtile_producer returning the SBUF tile the reducer writes.
    stream_ap: the DRAM-streamed operand used to size the DMA pools
    (defaults to kxn_ap).  kxn_pool: pre-created pool shared across calls
    so the next call's first tiles prefetch during the previous call.
    """
    nc = tc.nc
    if stream_ap is None:
        stream_ap = kxn_ap if kxn_cache is None else kxm_ap
    if kxm_cache is not None and kxn_cache is not None:
        num_bufs = 2
    else:
        num_bufs = k_pool_min_bufs(stream_ap, max_tile_size=max_k_tile)
    if kxm_cache is None:
        kxm_pool = ctx.enter_context(
            tc.tile_pool(name="kxm_pool", bufs=num_bufs + 4)
        )
    else:
        kxm_pool = None
    if kxn_cache is None:
        if kxn_pool is None:
            kxn_pool = ctx.enter_context(tc.tile_pool(name="kxn_pool", bufs=num_bufs))
    else:
        kxn_pool = None
    kxm_producer, kxm_shape = dma_from_dram_kxm(kxm_pool, kxm_ap, kxm_cache=kxm_cache)
    kxn_producer, kxn_shape = dma_from_dram_kxn(kxn_pool, kxn_ap, kxn_cache=kxn_cache)
    composable_matmul_tile_kernel(
        tc=tc,
        kxm_shape=kxm_shape,
        kxn_shape=kxn_shape,
        output_type=output_type if producer is None else None,
        kxm_producer=kxm_producer,
        kxn_producer=kxn_producer,
        mxn_subtile_reducer=reducer if reducer_override is None else reducer_override,
        mxn_consumer=consumer,
        mxn_subtile_producer=producer,
        psum_n_bufs=psum_n_bufs,
        MAX_K_TILE_SIZE=max_k_tile,
    )


def _build_graph(b1_zero=True, b2_zero=True):
    nc = bacc.Bacc(
        "TRN2",
        target_bir_lowering=False,
        debug=False,
        enable_asserts=False,
        num_devices=NCORES,
    )

    ADT = FP8 if USE_FP8 else BF16

    # ---- kernel I/O (per core) ----
    WBT = ADT if M3_FP8 else BF16
    xe = nc.dram_tensor("xe", [N, KEP], ADT, kind="ExternalInput")        # x_ext (replicated, padded)
    at8 = nc.dram_tensor("at8", [N, S], ADT, kind="ExternalInput")        # A.T[:, rows_c] (sharded)
    wcb = nc.dram_tensor("wcb", [KEP, HID], WBT, kind="ExternalInput")    # We_ext.T @ W1.T (replicated)
    w2 = nc.dram_tensor("w2", [HID, HID], ADT, kind="ExternalInput")      # W2.T * WSCALE (replicated)
    b1 = nc.dram_tensor("b1", [128, HID // 128], F32, kind="ExternalInput")  # b1 partition-major
    b2 = nc.dram_tensor("b2", [128, HID], F32, kind="ExternalInput")      # b2*WSCALE bcast (replicated)
    wc1 = nc.dram_tensor("wc1", [S, CHID], BF16, kind="ExternalInput")    # Wc1.T row-chunk (sharded)
    bc1 = nc.dram_tensor("bc1", [128, CHID // 128], F32, kind="ExternalInput")  # bc1 [128,16]
    wc2 = nc.dram_tensor("wc2", [128, CHID // 128], F32, kind="ExternalInput")  # Wc2 [128,16]
    res = nc.dram_tensor("res", [1, 1], F32, kind="ExternalOutput")       # final scalar (pre-bc2)

    # ---- internal DRAM ----
    # p = h1 @ (W2.T*64) is exchanged in column halves so each AllGather
    # overlaps with the production of the next half (M5) / consumption of
    # the previous one (M4).
    p_c = [nc.dram_tensor(f"pc{i}", [S, HQ], ADT) for i in range(NAG)]
    p_f = [
        nc.dram_tensor(f"pf{i}", [N, HQ], ADT, addr_space="Shared")
        for i in range(NAG)
    ]
    zb = [nc.dram_tensor(f"zb{i}", [1, CHID], F32) for i in range(NAG)]
    zf = [
        nc.dram_tensor(f"zf{i}", [1, CHID], F32, addr_space="Shared")
        for i in range(NAG)
    ]
    da = nc.dram_tensor("da", [1, 8], F32)              # launch-skew sync dummy
    df = nc.dram_tensor("df", [1, 8], F32, addr_space="Shared")

    MSUB = S // 128   # 4 m-subtiles in a 512-row tile
    NT = HID // 512   # 8 n-tiles of 512
    NTQ = NT // NAG   # n-tiles per AG half

    with tile.TileContext(nc) as tc:
        with contextlib.ExitStack() as octx:
            const = octx.enter_context(tc.tile_pool(name="const", bufs=1))
            b2_sb = (
                None if b2_zero else const.tile([128, HID], F32, name="b2_sb")
            )
            b1c_sb = const.tile([128, HID // 128], F32, name="b1c_sb")
            hm_parts = const.tile([128, MSUB, NT], F32, name="hm_parts")
            hm_sb = const.tile([128, MSUB], F32, name="hm_sb")
            hm_bf = const.tile([128, MSUB], BF16, name="hm_bf")
            nc.any.memset(hm_parts[:], 0.0)

            head = octx.enter_context(tc.tile_pool(name="head", bufs=1))
            CI = CHID // 128  # 16
            wc1_t = head.tile([128, MSUB, CHID], BF16, name="wc1_t")
            bc1_t = head.tile([128, CI], F32, name="bc1_t")
            wc2_t = head.tile([128, CI], F32, name="wc2_t")
            zp_t = [head.tile([1, CHID], F32, name=f"zp_t{i}") for i in range(NAG)]
            hm_i = [head.tile([128, MSUB], F32, name=f"hm_i{i}") for i in range(NAG)]
            hm_ib = [head.tile([128, MSUB], BF16, name=f"hm_ib{i}") for i in range(NAG)]
            z2_t = head.tile([128, CI], F32, name="z2_t")
            z2b_t = head.tile([128, CI], F32, name="z2b_t")
            zcol_t = head.tile([128, 1], F32, name="zcol_t")
            ones_t = head.tile([128, 1], F32, name="ones_t")
            r_t = head.tile([1, 1], F32, name="r_t")
            nc.any.memset(ones_t[:], 1.0)

            # persistent SBUF caches for chained-matmul intermediates
            cache = octx.enter_context(tc.tile_pool(name="cache", bufs=1))
            at8_c = cache.tile([128, N // 128, S], ADT, name="at8_c")
            # xe / tT_h live only until M1 completes; their pool closes
            # right after so the M5/M4 stream pools can reuse the space
            m1ctx = contextlib.ExitStack()
            m1pool = m1ctx.enter_context(tc.tile_pool(name="m1pool", bufs=1))
            xe_h = [
                m1pool.tile([128, N // 256, KEP], ADT, name=f"xe_h{i}")
                for i in range(2)
            ]
            at8_h = [
                m1pool.tile([128, N // 256, S], ADT, name=f"at8_h{i}")
                for i in range(2)
            ]
            tT_c = cache.tile([128, KEP // 128, S], WBT, name="tT_c")
            tT_h = m1pool.tile([128, KEP // 128, S], BF16, name="tT_h")
            h1T_c = cache.tile([128, HID // 128, S], ADT, name="h1T_c")
            # both M1 operands are fully cached via big contiguous DMAs
            # (streaming xe per-tile emits 128B descriptor rows -> ~5x slow);
            # chunked + interleaved so M1's first k-tile starts early
            wcb_c = cache.tile([128, KEP // 128, HID], WBT, name="wcb_c")
            at8_r = at8[:, :].rearrange("(po pi) n -> pi po n", pi=128)
            xe_r = xe[:, :].rearrange("(po pi) n -> pi po n", pi=128)
            wcb_r = wcb[:, :].rearrange("(po pi) n -> pi po n", pi=128)
            HK = N // 256  # k-subtiles per xe half (16)
            # M1's operands go to half-tiles so its first matmuls depend on
            # only ~2.3MB of DMA (dep tracking is tile-granular); M4's full
            # at8_c cache is re-read from DRAM later, hidden under M3/M5
            for po in range(0, N // 256, 4):
                nc.sync.dma_start(out=at8_h[0][:, po : po + 4, :], in_=at8_r[:, po : po + 4, :])
                nc.sync.dma_start(out=xe_h[0][:, po : po + 4, :], in_=xe_r[:, po : po + 4, :])
            for po in range(N // 256, N // 128, 4):
                nc.sync.dma_start(
                    out=at8_h[1][:, po - HK : po - HK + 4, :],
                    in_=at8_r[:, po : po + 4, :],
                )
                nc.sync.dma_start(
                    out=xe_h[1][:, po - HK : po - HK + 4, :],
                    in_=xe_r[:, po : po + 4, :],
                )
            # wcb cached too (M3's weight side): issued after the M1 operands
            # so it lands just as M1 finishes
            for po in range(KEP // 128):
                nc.sync.dma_start(
                    out=wcb_c[:, po : po + 1, :], in_=wcb_r[:, po : po + 1, :]
                )
            noop = lambda nc_, sbuf, md: None

            # dummy AllReduce fired first: absorbs the multi-core launch skew
            # on the collective engine while the PE is busy with M1/M3/M5, so
            # the p AllGathers later only see compute drift
            sync_t = head.tile([1, 8], F32, name="sync_t")
            nc.any.memset(sync_t[:], 0.0)
            nc.sync.dma_start(out=da[:, :], in_=sync_t[:, :])
            nc.gpsimd.collective_compute(
                "AllReduce",
                mybir.AluOpType.add,
                ins=[da[:, :].opt()],
                outs=[df[:, :].opt()],
                replica_groups=[list(range(NCORES))],
            )

            # M1: tT = (A_c @ x_ext).T = xe.T @ A_c.T        [KE, S]
            # run as two source-half matmuls so the first starts when only
            # half the operand caches have landed (tile-granular DMA deps);
            # halves are combined with a cheap DVE add into tT_c
            for hi in range(2):
                with contextlib.ExitStack() as ctx:
                    _matmul_custom(
                        ctx, tc, xe_h[hi][:, :, :],
                        at8_h[hi][:, :, :],
                        scalar_copyback(), noop,
                        kxm_cache=xe_h[hi][:, :, :],
                        kxn_cache=at8_h[hi][:, :, :],
                        producer=(
                            (lambda nc_, md: tT_h[
                                :,
                                md.m_subtiles * md.m_tile_idx : md.m_subtiles
                                * (md.m_tile_idx + 1),
                                :,
                            ])
                            if hi == 0
                            else None
                        ),
                        output_type=None if hi == 0 else F32,
                        reducer_override=(
                            None
                            if hi == 0
                            else _m1_combine_reducer(tT_h, tT_c)
                        ),
                        max_k_tile=512,
                    )
            m1ctx.close()
            # M4's at8 cache loads now, overlapped under M3/M5
            for po in range(0, N // 128, 8):
                nc.sync.dma_start(
                    out=at8_c[:, po : po + 8, :], in_=at8_r[:, po : po + 8, :]
                )
            # b1 prefetch after M1 so it doesn't starve M1's tiles in the
            # DMA queues (scheduler priority = trace order)
            if not b1_zero:
                nc.sync.dma_start(out=b1c_sb[:, :], in_=b1[:, :])

            # M3: h1T = relu((We_ext.T W1.T).T @ t.T + b1)   [HID, S]
            # feature-major so h1T is directly the kxm cache for M5
            with contextlib.ExitStack() as ctx:
                _matmul_custom(
                    ctx, tc, wcb_c[:, :, :], tT_c[:, :, :],
                    _m3_reducer(nc, None if b1_zero else b1c_sb),
                    noop,
                    kxm_cache=wcb_c[:, :, :],
                    kxn_cache=tT_c[:, :, :],
                    producer=lambda nc_, md: h1T_c[
                        :, MSUB * md.m_tile_idx : MSUB * (md.m_tile_idx + 1), :
                    ],
                    # 256 keeps K_SUBTILES even so fp8 DoubleRow engages
                    max_k_tile=256 if M3_FP8 else 512,
                )

            # M5 halves: p[:, half] = h1T.T @ w2[:, half]    [S, HQ] fp8
            # entirely local (W2 replicated) -- this is the PE work that
            # hides the collective rendezvous + AllGather; each half's AG
            # fires as soon as the half is in DRAM.  With M3_FP8 both h1T
            # and w2 carry a WSCALE factor, so the eviction divides one out.
            m5_evict = (
                scalar_scale(1.0 / WCB_SCALE) if M3_FP8 else scalar_copyback()
            )
            m5_kxn_pool = octx.enter_context(
                tc.tile_pool(name="m5_kxn_pool", bufs=6)
            )
            for i in range(NAG):
                cons = dma_to_dram_mxn(p_c[i][:, :])
                with contextlib.ExitStack() as ctx:
                    _matmul_custom(
                        ctx, tc, h1T_c[:, :, :], w2[:, i * HQ : (i + 1) * HQ],
                        m5_evict, cons,
                        output_type=ADT,
                        kxm_cache=h1T_c[:, :, :],
                        max_k_tile=2048,
                        kxn_pool=m5_kxn_pool,
                    )
                nc.gpsimd.collective_compute(
                    "AllGather",
                    mybir.AluOpType.bypass,
                    ins=[p_c[i][:, :].opt()],
                    outs=[p_f[i][:, :].opt()],
                    replica_groups=[list(range(NCORES))],
                )
            # b2 prefetch (needed by M4's reducer) after the AG triggers so
            # the p bounce writes aren't queued behind it
            if not b2_zero:
                nc.sync.dma_start(out=b2_sb[:, :], in_=b2[:, :])

            # M4 chunks: aggr2[:, chunk] = at8.T @ p_full[:, chunk]  [S, HQ]
            # relu+b2 fused into the eviction; row-sums stream into hm_parts.
            # w2 was pre-scaled by WSCALE and b2 holds WSCALE*b2, so the
            # accumulated sums are WSCALE*h2; the hm normalization divides
            # it back out.
            m4_kxn_pool = octx.enter_context(
                tc.tile_pool(name="m4_kxn_pool", bufs=6)
            )
            # head psum pool coexists with M4's (psum_n_bufs=1 there):
            # 4 + 3 banks <= 8
            hpsum = octx.enter_context(
                tc.tile_pool(name="hpsum", bufs=2, space="PSUM")
            )
            NB = CHID // 512  # 4 zp column blocks
            for i in range(NAG):
                with contextlib.ExitStack() as ctx:
                    _matmul_custom(
                        ctx, tc, at8_c[:, :, :], p_f[i][:, :],
                        _m4_reducer(
                            nc, None if b2_zero else b2_sb, hm_parts, i * NTQ
                        ),
                        noop,
                        # relu output is scratch (only the fused row-sum is
                        # kept), so fp8 minimizes its SBUF footprint
                        output_type=FP8,
                        psum_n_bufs=1,
                        kxm_cache=at8_c[:, :, :],
                        # 512-k tiles: the first p_f DMA after each
                        # AllGather is 256KB instead of 1MB, shortening the
                        # post-collective restart latency
                        max_k_tile=512,
                        kxn_pool=m4_kxn_pool,
                    )
                if i == 0:
                    nc.sync.dma_start(
                        out=wc1_t[:, :, :],
                        in_=wc1[:, :].rearrange("(po pi) n -> pi po n", pi=128),
                    )
                    nc.sync.dma_start(out=bc1_t[:, :], in_=bc1[:, :])
                    nc.sync.dma_start(out=wc2_t[:, :], in_=wc2[:, :])
                # split-z: this chunk's hm contribution -> zp_i -> AllReduce,
                # so chunk 0's AR rides under chunk 1's aggregation and only
                # the last AR sits on the tail
                nc.vector.tensor_reduce(
                    out=hm_i[i][:, :],
                    in_=hm_parts[:, :, i * NTQ : (i + 1) * NTQ],
                    axis=mybir.AxisListType.X, op=mybir.AluOpType.add,
                )
                nc.vector.tensor_scalar_mul(
                    hm_i[i][:, :], hm_i[i][:, :],
                    1.0 / (HID * (WSCALE if USE_FP8 else 1.0)),
                )
                nc.vector.tensor_copy(out=hm_ib[i][:, :], in_=hm_i[i][:, :])
                for j in range(NB):
                    psj = hpsum.tile([128, 512], F32, name="zpps")
                    for ko in range(MSUB):
                        nc.tensor.matmul(
                            psj[0:1, :],
                            hm_ib[i][:, ko : ko + 1],
                            wc1_t[:, ko, 512 * j : 512 * (j + 1)],
                            start=(ko == 0),
                            stop=(ko == MSUB - 1),
                        )
                    nc.vector.tensor_copy(
                        out=zp_t[i][:, 512 * j : 512 * (j + 1)], in_=psj[0:1, :]
                    )
                nc.sync.dma_start(out=zb[i][:, :], in_=zp_t[i][:, :])
                nc.gpsimd.collective_compute(
                    "AllReduce",
                    mybir.AluOpType.add,
                    ins=[zb[i][:, :].opt()],
                    outs=[zf[i][:, :].opt()],
                    replica_groups=[list(range(NCORES))],
                )
            # epilogue on z viewed as [128, 16] so the DVE ops use all lanes
            psr = hpsum.tile([128, 512], F32, name="zpps")
            nc.sync.dma_start(
                out=z2_t[:, :], in_=zf[0][:, :].rearrange("o (p i) -> p (o i)", p=128)
            )
            nc.sync.dma_start(
                out=z2b_t[:, :], in_=zf[1][:, :].rearrange("o (p i) -> p (o i)", p=128)
            )
            nc.vector.tensor_add(out=z2_t[:, :], in0=z2_t[:, :], in1=z2b_t[:, :])
            nc.vector.tensor_add(out=z2_t[:, :], in0=z2_t[:, :], in1=bc1_t[:, :])
            nc.vector.tensor_scalar_max(z2_t[:, :], z2_t[:, :], 0.0)
            nc.vector.tensor_mul(out=z2_t[:, :], in0=z2_t[:, :], in1=wc2_t[:, :])
            nc.vector.tensor_reduce(
                out=zcol_t[:, :], in_=z2_t[:, :],
                axis=mybir.AxisListType.X, op=mybir.AluOpType.add,
            )
            # cross-partition sum via a 128x1 ones matmul
            nc.tensor.matmul(
                psr[0:1, 0:1], ones_t[:, 0:1], zcol_t[:, 0:1], start=True, stop=True
            )
            nc.vector.tensor_copy(out=r_t[:, :], in_=psr[0:1, 0:1])
            nc.sync.dma_start(out=res[:, :], in_=r_t[:, :])

    nc.compile()
    nc.m = get_hw_module(nc.m)
    return nc


def get_compiled(b1_zero=True, b2_zero=True):
    key = (b1_zero, b2_zero)
    if key not in _COMPILED:
        _COMPILED[key] = _build_graph(*key)
    return _COMPILED[key]


def _bf16(a):
    return np.ascontiguousarray(np.asarray(a, dtype=np.float32)).astype(ml_dtypes.bfloat16)


def _f32(a):
    return np.ascontiguousarray(np.asarray(a, dtype=np.float32))


_NP_FP8 = mybir.dt.np(FP8)


def _adt(a):
    """Convert to the aggregation dtype (fp8 or bf16)."""
    a = np.ascontiguousarray(np.asarray(a, dtype=np.float32))
    return a.astype(_NP_FP8 if USE_FP8 else ml_dtypes.bfloat16)


def make_in_maps(x, edge_index, W_embed, b_embed, W1, b1, W2, b2, Wc1, bc1, Wc2, bc2):
    x = _f32(x)
    ei = np.asarray(edge_index).astype(np.int64)
    # adjacency counts, transposed: AT[src, dst] = #edges src->dst
    counts = np.bincount(ei[1] * N + ei[0], minlength=N * N).astype(np.float32)
    AT = counts.reshape(N, N)

    # padded to KEP so M1 computes the tT DoubleRow-pad rows as real zeros
    x_ext = np.zeros((N, KEP), np.float32)
    x_ext[:, :IN_DIM] = x
    x_ext[:, IN_DIM] = 1.0

    we_ext = np.zeros((KEP, HID), np.float32)
    we_ext[:IN_DIM] = _f32(W_embed).T
    we_ext[IN_DIM] = _f32(b_embed)
    # layer-1 transform is low-rank: fold We_ext.T @ W1.T on the host
    wcb_full = we_ext @ _f32(W1).T
    if M3_FP8:
        # scale into e4m3's normal range; h1T then carries WCB_SCALE and
        # the p eviction divides it back out
        wcb_np = _adt(wcb_full * WCB_SCALE)
    else:
        wcb_np = _bf16(wcb_full)

    xe_np = _adt(x_ext)
    at8_np = _adt(AT)
    wmul = WSCALE if USE_FP8 else 1.0
    w2_np = _adt(_f32(W2).T * wmul) if USE_FP8 else _bf16(_f32(W2).T)
    # b1 per-partition layout for the feature-major h1T eviction (h1T
    # carries the WCB_SCALE factor, so b1 must too)
    b1s = _f32(b1) * (WCB_SCALE if M3_FP8 else 1.0)
    b1c_np = _f32(np.ascontiguousarray(b1s.reshape(HID // 128, 128).T))
    b2s = _f32(b2) * (WSCALE if USE_FP8 else 1.0)
    b2_np = _f32(np.broadcast_to(b2s, (128, HID)))
    wc1T = _bf16(_f32(Wc1).T)  # [HID(nodes), CHID] bf16
    wc2_row = _f32(Wc2).reshape(128, CHID // 128)
    bc1_full = _f32(bc1).reshape(128, CHID // 128)

    in_maps = []
    for c in range(NCORES):
        rows = slice(S * c, S * (c + 1))
        in_maps.append(
            {
                "xe": xe_np,
                "wcb": wcb_np,
                "at8": np.ascontiguousarray(at8_np[:, rows]),
                "w2": w2_np,
                "b1": b1c_np,
                "b2": b2_np,
                "wc1": np.ascontiguousarray(wc1T[rows, :]),
                "bc1": bc1_full,
                "wc2": wc2_row,
            }
        )
    return in_maps


def kernel(**inputs):
    b1_zero = not np.any(np.asarray(inputs["b1"], dtype=np.float32))
    b2_zero = not np.any(np.asarray(inputs["b2"], dtype=np.float32))
    nc = get_compiled(b1_zero, b2_zero)
    in_maps = make_in_maps(**inputs)
    bres = run_bass_kernel_spmd(nc, in_maps, core_ids=list(range(NCORES)))
    val = np.float32(bres.results[0]["res"][0, 0])
    bc2 = np.asarray(inputs["bc2"], dtype=np.float32).reshape(-1)
    out = np.asarray(val + bc2[0], dtype=np.float32).reshape(())
    return out


# revision 43
# speedup vs baseline: 1.0165x; 1.0165x over previous
"""Trainium2 Bass kernel for a 2-layer message-passing GNN (BaselineGNN).

Reference computation (N=4096 nodes, IN=512, HID=4096, E=65536 edges):
    h   = x @ We.T + be                                   [N, HID]
    for W, b in ((W1, b1), (W2, b2)):
        aggr = segment_sum(h[col], row)                   [N, HID]
        h    = relu(aggr @ W.T + b)
    hm  = mean(h, axis=1)                                 [N]
    z   = relu(hm @ Wc1.T + bc1)                          [HID//2]
    out = (z @ Wc2.T + bc2).squeeze(-1)                   scalar

Strategy (8 NeuronCores, node-parallel, collective-hiding reorder):
  * segment_sum == A @ h with A the [N, N] adjacency-count matrix (0.4%
    dense).  A's entries are small integer counts -> exactly representable
    in fp8-e4m3, so aggregation runs as a dense TensorEngine matmul.
  * Nodes are sharded: core c owns rows 512c..512c+512.
  * Layer 1 is low-rank through the embed bottleneck:
        h1_c = relu((A_c @ x_ext) @ (We_ext.T @ W1.T) + b1)
    with the weight product folded on the host (x_ext carries a ones
    column, We_ext.T a b_embed row).
  * Layer 2 is REORDERED vs the naive form to hide the collective:
        h2 = relu(A @ h1 @ W2.T + b2) = relu(A @ (h1 @ W2.T) + b2)
    Each core computes p_c = h1_c @ W2.T *locally* (W2 replicated) BEFORE
    any cross-core exchange -- ~150us of PE work that overlaps the
    first-collective rendezvous/launch-skew barrier AND the p AllGather.
    p is AllGathered in column halves; the final aggregation
    aggr2 = A_c @ p_full then runs per half with relu+b2+row-sum fused
    into the PSUM eviction (ScalarE accum_out) -> hm directly.  No
    second weight matmul after the collective, so the PE pipeline is
    near-continuous: M1 -> M3 -> M5 (local, hides AG) -> M4 -> head.
  * Pipeline per core: M1 tT=(A_c@x_ext).T [fp8 DR, split into source
    halves so it starts before the operand caches fully land] -> M3 h1T
    [bf16, feature-major so h1T is the kxm cache for M5] -> M5 halves
    p_c = h1T.T @ (W2.T*16) [fp8 DR] -> AllGather half (fp8) ->
    M4 halves aggr2 = at8.T @ p_full [fp8 DR] with relu+row-sum fused
    eviction -> per-half partial z = Wc1@hm_half AllReduced early so only
    the last AR sits on the tail -> epilogue.
  * Chained matmuls hand intermediates over in SBUF caches (tT_c, h1T_c,
    at8_c) via mxn_subtile_producer / kxm/kxn_cache.
  * fp8 operands that are not exact (x, h1, p, W2) rely on fp32 PSUM
    accumulation and the mean-pool/classifier averaging to wash the 6%
    element-level rounding down to ~4e-4 relative error on the scalar
    output; W2 is pre-scaled by 16 to sit in e4m3's normal range (the
    hm normalization divides it back out).  The classifier runs bf16/fp32.
  * A dummy AllReduce issued first absorbs the multi-core launch skew on
    the collective engine while the PE computes M1/M3/M5.

Everything is expressed "transposed-free": every matmul is
mxn = kxm.T @ kxn with operands stored so no transposes are ever needed.
"""

import contextlib

import numpy as np
import ml_dtypes

import concourse.bass as bass
import concourse.mybir as mybir
import concourse.tile as tile
from concourse import bacc
from concourse.bass_interp import get_hw_module
from concourse.bass_utils import run_bass_kernel_spmd
from concourse.kernels.tile_matmul import (
    composable_matmul_tile_kernel,
    dma_from_dram_kxm,
    dma_from_dram_kxn,
    dma_to_dram_mxn,
    k_pool_min_bufs,
    scalar_copyback,
    scalar_scale,
)

N = 4096          # nodes
IN_DIM = 512
HID = 4096
NCORES = 8
S = N // NCORES           # nodes per core (512)
KE = 640                  # extended embed contraction (512 + 1 ones col, padded to 5*128)
CHID = HID // 2           # classifier hidden (2048)

BF16 = mybir.dt.bfloat16
F32 = mybir.dt.float32
FP8 = mybir.dt.float8e4

USE_FP8 = True
# p = h1 @ (W2.T * WSCALE) is materialized in fp8-e4m3 (range +-448):
# p_true has absmax ~12, so WSCALE=16 centers it (~200 max) with margin.
WSCALE = 16.0
# layer-1 path in fp8 DoubleRow: wcb is scaled by WCB_SCALE (its entries
# are ~9e-3, below e4m3's normal range), h1T carries the factor (absmax
# 15.6*16=250 < 448; x32 would clip), and the p eviction divides it back
# out so p stays WSCALE-scaled.
M3_FP8 = False
WCB_SCALE = 16.0
KEP = 768 if M3_FP8 else KE   # KE padded to a DoubleRow-even k (6*128)

NAG = 2                   # AllGather column-chunk count
HQ = HID // NAG

_COMPILED = {}


def _m4_reducer(nc_b, bias_sb, accum, n_off):
    """PSUM->SBUF evict for the final aggregation:
    sbuf = relu(psum + b2[n_slice]); row-sums stream into accum.

    bias_sb: [128, HID] SBUF tile (b2 broadcast across partitions), or None
    when b2 is all-zero.  accum: [128, MSUB, NT] fp32; the relu
    row-sum-accumulates into accum[:, m_subtile, n_off + n_tile].
    """

    def _reducer(nc, psum, sbuf, md):
        src_ap = psum[:, : md.n_slice_size]
        ni = n_off + md.n_tile_idx * md.n_subtiles + md.n_subtile_idx
        if bias_sb is not None:
            start = (
                n_off * md.n_tile
                + md.n_tile_idx * md.n_tile
                + md.n_subtile_idx * md.n_subtile
            )
            nc.vector.tensor_add(
                out=sbuf[:, :, : md.n_slice_size],
                in0=src_ap,
                in1=bias_sb[:, start : start + md.n_slice_size],
            )
            src_ap = sbuf
        nc.scalar.activation(
            out=sbuf,
            in_=src_ap,
            func=mybir.ActivationFunctionType.Relu,
            accum_out=accum[:, md.m_subtile_idx, ni : ni + 1],
        )

    return _reducer


def _m1_combine_reducer(tT_h, tT_c):
    """Second M1 half eviction: tT_c = fp(psum + tT_h) (combine k-halves)."""

    def _reducer(nc, psum, sbuf, md):
        mi = md.m_subtiles * md.m_tile_idx + md.m_subtile_idx
        nc.vector.tensor_add(
            out=tT_c[:, mi : mi + 1, : md.n_slice_size],
            in0=psum[:, : md.n_slice_size],
            in1=tT_h[:, mi : mi + 1, : md.n_slice_size],
        )

    return _reducer


def _m3_reducer(nc_b, b1c_sb):
    """PSUM->SBUF evict for h1T: sbuf = relu(psum + b1[m_slice]).

    h1T is feature-major, so the layer-1 bias is per-partition:
    b1c_sb is [128, HID//128] with b1c[p, m] = b1[m*128 + p], or None
    when b1 is all-zero.
    """

    def _reducer(nc, psum, sbuf, md):
        if b1c_sb is None:
            nc.scalar.activation(
                out=sbuf, in_=psum, func=mybir.ActivationFunctionType.Relu
            )
        else:
            mi = md.m_tile_idx * md.m_subtiles + md.m_subtile_idx
            nc.vector.tensor_add(
                out=sbuf[:, :, : md.n_slice_size],
                in0=psum[:, : md.n_slice_size],
                in1=b1c_sb[:, mi : mi + 1].to_broadcast(
                    [128, 1, md.n_slice_size]
                ),
            )
            nc.vector.tensor_scalar_max(sbuf, sbuf, 0.0)

    return _reducer


def _matmul_custom(ctx, tc, kxm_ap, kxn_ap, reducer, consumer, output_type=F32,
                   psum_n_bufs=2, kxm_cache=None, kxn_cache=None, producer=None,
                   max_k_tile=512, stream_ap=None, kxn_pool=None,
                   reducer_override=None):
    """composable matmul with custom psum reducer / mxn consumer / SBUF caches.

    kxm_cache / kxn_cache: [128, K//128, M-or-N] SBUF tiles already holding
    the operand (no DMA is issued for that side).  producer: optional
    mxn_subtile_producer returning the SBUF tile the reducer writes.
    stream_ap: the DRAM-streamed operand used to size the DMA pools
    (defaults to kxn_ap).  kxn_pool: pre-created pool shared across calls
    so the next call's first tiles prefetch during the previous call.
    """
    nc = tc.nc
    if stream_ap is None:
        stream_ap = kxn_ap if kxn_cache is None else kxm_ap
    if kxm_cache is not None and kxn_cache is not None:
        num_bufs = 2
    else:
        num_bufs = k_pool_min_bufs(stream_ap, max_tile_size=max_k_tile)
    if kxm_cache is None:
        kxm_pool = ctx.enter_context(
            tc.tile_pool(name="kxm_pool", bufs=num_bufs + 4)
        )
    else:
        kxm_pool = None
    if kxn_cache is None:
        if kxn_pool is None:
            kxn_pool = ctx.enter_context(tc.tile_pool(name="kxn_pool", bufs=num_bufs))
    else:
        kxn_pool = None
    kxm_producer, kxm_shape = dma_from_dram_kxm(kxm_pool, kxm_ap, kxm_cache=kxm_cache)
    kxn_producer, kxn_shape = dma_from_dram_kxn(kxn_pool, kxn_ap, kxn_cache=kxn_cache)
    composable_matmul_tile_kernel(
        tc=tc,
        kxm_shape=kxm_shape,
        kxn_shape=kxn_shape,
        output_type=output_type if producer is None else None,
        kxm_producer=kxm_producer,
        kxn_producer=kxn_producer,
        mxn_subtile_reducer=reducer if reducer_override is None else reducer_override,
        mxn_consumer=consumer,
        mxn_subtile_producer=producer,
        psum_n_bufs=psum_n_bufs,
        MAX_K_TILE_SIZE=max_k_tile,
    )


def _build_graph(b1_zero=True, b2_zero=True):
    nc = bacc.Bacc(
        "TRN2",
        target_bir_lowering=False,
        debug=False,
        enable_asserts=False,
        num_devices=NCORES,
    )

    ADT = FP8 if USE_FP8 else BF16

    # ---- kernel I/O (per core) ----
    WBT = ADT if M3_FP8 else BF16
    xe = nc.dram_tensor("xe", [N, KEP], ADT, kind="ExternalInput")        # x_ext (replicated, padded)
    at8 = nc.dram_tensor("at8", [N, S], ADT, kind="ExternalInput")        # A.T[:, rows_c] (sharded)
    wcb = nc.dram_tensor("wcb", [KEP, HID], WBT, kind="ExternalInput")    # We_ext.T @ W1.T (replicated)
    w2 = nc.dram_tensor("w2", [HID, HID], ADT, kind="ExternalInput")      # W2.T * WSCALE (replicated)
    b1 = nc.dram_tensor("b1", [128, HID // 128], F32, kind="ExternalInput")  # b1 partition-major
    b2 = nc.dram_tensor("b2", [128, HID], F32, kind="ExternalInput")      # b2*WSCALE bcast (replicated)
    wc1 = nc.dram_tensor("wc1", [S, CHID], BF16, kind="ExternalInput")    # Wc1.T row-chunk (sharded)
    bc1 = nc.dram_tensor("bc1", [128, CHID // 128], F32, kind="ExternalInput")  # bc1 [128,16]
    wc2 = nc.dram_tensor("wc2", [128, CHID // 128], F32, kind="ExternalInput")  # Wc2 [128,16]
    res = nc.dram_tensor("res", [1, 1], F32, kind="ExternalOutput")       # final scalar (pre-bc2)

    # ---- internal DRAM ----
    # p = h1 @ (W2.T*64) is exchanged in column halves so each AllGather
    # overlaps with the production of the next half (M5) / consumption of
    # the previous one (M4).
    p_c = [nc.dram_tensor(f"pc{i}", [S, HQ], ADT) for i in range(NAG)]
    p_f = [
        nc.dram_tensor(f"pf{i}", [N, HQ], ADT, addr_space="Shared")
        for i in range(NAG)
    ]
    zb = [nc.dram_tensor(f"zb{i}", [1, CHID], F32) for i in range(NAG)]
    zf = [
        nc.dram_tensor(f"zf{i}", [1, CHID], F32, addr_space="Shared")
        for i in range(NAG)
    ]
    da = nc.dram_tensor("da", [1, 8], F32)              # launch-skew sync dummy
    df = nc.dram_tensor("df", [1, 8], F32, addr_space="Shared")

    MSUB = S // 128   # 4 m-subtiles in a 512-row tile
    NT = HID // 512   # 8 n-tiles of 512
    NTQ = NT // NAG   # n-tiles per AG half

    with tile.TileContext(nc) as tc:
        with contextlib.ExitStack() as octx:
            const = octx.enter_context(tc.tile_pool(name="const", bufs=1))
            b2_sb = (
                None if b2_zero else const.tile([128, HID], F32, name="b2_sb")
            )
            b1c_sb = const.tile([128, HID // 128], F32, name="b1c_sb")
            hm_parts = const.tile([128, MSUB, NT], F32, name="hm_parts")
            hm_sb = const.tile([128, MSUB], F32, name="hm_sb")
            hm_bf = const.tile([128, MSUB], BF16, name="hm_bf")
            nc.any.memset(hm_parts[:], 0.0)

            head = octx.enter_context(tc.tile_pool(name="head", bufs=1))
            CI = CHID // 128  # 16
            wc1_t = head.tile([128, MSUB, CHID], BF16, name="wc1_t")
            bc1_t = head.tile([128, CI], F32, name="bc1_t")
            wc2_t = head.tile([128, CI], F32, name="wc2_t")
            zp_t = [head.tile([1, CHID], F32, name=f"zp_t{i}") for i in range(NAG)]
            hm_i = [head.tile([128, MSUB], F32, name=f"hm_i{i}") for i in range(NAG)]
            hm_ib = [head.tile([128, MSUB], BF16, name=f"hm_ib{i}") for i in range(NAG)]
            z2_t = head.tile([128, CI], F32, name="z2_t")
            z2b_t = head.tile([128, CI], F32, name="z2b_t")
            zcol_t = head.tile([128, 1], F32, name="zcol_t")
            ones_t = head.tile([128, 1], F32, name="ones_t")
            r_t = head.tile([1, 1], F32, name="r_t")
            nc.any.memset(ones_t[:], 1.0)

            # persistent SBUF caches for chained-matmul intermediates
            cache = octx.enter_context(tc.tile_pool(name="cache", bufs=1))
            at8_c = cache.tile([128, N // 128, S], ADT, name="at8_c")
            # xe / tT_h live only until M1 completes; their pool closes
            # right after so the M5/M4 stream pools can reuse the space
            m1ctx = contextlib.ExitStack()
            m1pool = m1ctx.enter_context(tc.tile_pool(name="m1pool", bufs=1))
            xe_h = [
                m1pool.tile([128, N // 256, KEP], ADT, name=f"xe_h{i}")
                for i in range(2)
            ]
            at8_h = [
                m1pool.tile([128, N // 256, S], ADT, name=f"at8_h{i}")
                for i in range(2)
            ]
            tT_c = cache.tile([128, KEP // 128, S], WBT, name="tT_c")
            tT_h = m1pool.tile([128, KEP // 128, S], BF16, name="tT_h")
            h1T_c = cache.tile([128, HID // 128, S], ADT, name="h1T_c")
            # both M1 operands are fully cached via big contiguous DMAs
            # (streaming xe per-tile emits 128B descriptor rows -> ~5x slow);
            # chunked + interleaved so M1's first k-tile starts early
            wcb_c = cache.tile([128, KEP // 128, HID], WBT, name="wcb_c")
            at8_r = at8[:, :].rearrange("(po pi) n -> pi po n", pi=128)
            xe_r = xe[:, :].rearrange("(po pi) n -> pi po n", pi=128)
            wcb_r = wcb[:, :].rearrange("(po pi) n -> pi po n", pi=128)
            HK = N // 256  # k-subtiles per xe half (16)
            # M1's operands go to half-tiles so its first matmuls depend on
            # only ~2.3MB of DMA (dep tracking is tile-granular); M4's full
            # at8_c cache is re-read from DRAM later, hidden under M3/M5
            for po in range(0, N // 256, 4):
                nc.sync.dma_start(out=at8_h[0][:, po : po + 4, :], in_=at8_r[:, po : po + 4, :])
                nc.sync.dma_start(out=xe_h[0][:, po : po + 4, :], in_=xe_r[:, po : po + 4, :])
            for po in range(N // 256, N // 128, 4):
                nc.sync.dma_start(
                    out=at8_h[1][:, po - HK : po - HK + 4, :],
                    in_=at8_r[:, po : po + 4, :],
                )
                nc.sync.dma_start(
                    out=xe_h[1][:, po - HK : po - HK + 4, :],
                    in_=xe_r[:, po : po + 4, :],
                )
            # wcb cached too (M3's weight side): issued after the M1 operands
            # so it lands just as M1 finishes
            for po in range(KEP // 128):
                nc.sync.dma_start(
                    out=wcb_c[:, po : po + 1, :], in_=wcb_r[:, po : po + 1, :]
                )
            noop = lambda nc_, sbuf, md: None

            # dummy AllReduce fired first: absorbs the multi-core launch skew
            # on the collective engine while the PE is busy with M1/M3/M5, so
            # the p AllGathers later only see compute drift
            sync_t = head.tile([1, 8], F32, name="sync_t")
            nc.any.memset(sync_t[:], 0.0)
            nc.sync.dma_start(out=da[:, :], in_=sync_t[:, :])
            nc.gpsimd.collective_compute(
                "AllReduce",
                mybir.AluOpType.add,
                ins=[da[:, :].opt()],
                outs=[df[:, :].opt()],
                replica_groups=[list(range(NCORES))],
            )

            # M1: tT = (A_c @ x_ext).T = xe.T @ A_c.T        [KE, S]
            # run as two source-half matmuls so the first starts when only
            # half the operand caches have landed (tile-granular DMA deps);
            # halves are combined with a cheap DVE add into tT_c
            for hi in range(2):
                with contextlib.ExitStack() as ctx:
                    _matmul_custom(
                        ctx, tc, xe_h[hi][:, :, :],
                        at8_h[hi][:, :, :],
                        scalar_copyback(), noop,
                        kxm_cache=xe_h[hi][:, :, :],
                        kxn_cache=at8_h[hi][:, :, :],
                        producer=(
                            (lambda nc_, md: tT_h[
                                :,
                                md.m_subtiles * md.m_tile_idx : md.m_subtiles
                                * (md.m_tile_idx + 1),
                                :,
                            ])
                            if hi == 0
                            else None
                        ),
                        output_type=None if hi == 0 else F32,
                        reducer_override=(
                            None
                            if hi == 0
                            else _m1_combine_reducer(tT_h, tT_c)
                        ),
                        max_k_tile=512,
                    )
            m1ctx.close()
            # b1 prefetch after M1 so it doesn't starve M1's tiles in the
            # DMA queues (scheduler priority = trace order)
            if not b1_zero:
                nc.sync.dma_start(out=b1c_sb[:, :], in_=b1[:, :])

            # M3: h1T = relu((We_ext.T W1.T).T @ t.T + b1)   [HID, S]
            # feature-major so h1T is directly the kxm cache for M5
            with contextlib.ExitStack() as ctx:
                _matmul_custom(
                    ctx, tc, wcb_c[:, :, :], tT_c[:, :, :],
                    _m3_reducer(nc, None if b1_zero else b1c_sb),
                    noop,
                    kxm_cache=wcb_c[:, :, :],
                    kxn_cache=tT_c[:, :, :],
                    producer=lambda nc_, md: h1T_c[
                        :, MSUB * md.m_tile_idx : MSUB * (md.m_tile_idx + 1), :
                    ],
                    # 256 keeps K_SUBTILES even so fp8 DoubleRow engages
                    max_k_tile=256 if M3_FP8 else 512,
                )

            # M4's at8 cache loads here, after M3's operands, so it fills
            # DMA idle slots under M3/M5 without delaying wcb
            for po in range(0, N // 128, 8):
                nc.sync.dma_start(
                    out=at8_c[:, po : po + 8, :], in_=at8_r[:, po : po + 8, :]
                )

            # M5 halves: p[:, half] = h1T.T @ w2[:, half]    [S, HQ] fp8
            # entirely local (W2 replicated) -- this is the PE work that
            # hides the collective rendezvous + AllGather; each half's AG
            # fires as soon as the half is in DRAM.  With M3_FP8 both h1T
            # and w2 carry a WSCALE factor, so the eviction divides one out.
            m5_evict = (
                scalar_scale(1.0 / WCB_SCALE) if M3_FP8 else scalar_copyback()
            )
            m5_kxn_pool = octx.enter_context(
                tc.tile_pool(name="m5_kxn_pool", bufs=6)
            )
            for i in range(NAG):
                cons = dma_to_dram_mxn(p_c[i][:, :])
                with contextlib.ExitStack() as ctx:
                    _matmul_custom(
                        ctx, tc, h1T_c[:, :, :], w2[:, i * HQ : (i + 1) * HQ],
                        m5_evict, cons,
                        output_type=ADT,
                        kxm_cache=h1T_c[:, :, :],
                        max_k_tile=2048,
                        kxn_pool=m5_kxn_pool,
                    )
                nc.gpsimd.collective_compute(
                    "AllGather",
                    mybir.AluOpType.bypass,
                    ins=[p_c[i][:, :].opt()],
                    outs=[p_f[i][:, :].opt()],
                    replica_groups=[list(range(NCORES))],
                )
            # b2 prefetch (needed by M4's reducer) after the AG triggers so
            # the p bounce writes aren't queued behind it
            if not b2_zero:
                nc.sync.dma_start(out=b2_sb[:, :], in_=b2[:, :])

            # M4 chunks: aggr2[:, chunk] = at8.T @ p_full[:, chunk]  [S, HQ]
            # relu+b2 fused into the eviction; row-sums stream into hm_parts.
            # w2 was pre-scaled by WSCALE and b2 holds WSCALE*b2, so the
            # accumulated sums are WSCALE*h2; the hm normalization divides
            # it back out.
            m4_kxn_pool = octx.enter_context(
                tc.tile_pool(name="m4_kxn_pool", bufs=6)
            )
            # head psum pool coexists with M4's (psum_n_bufs=1 there):
            # 4 + 3 banks <= 8
            hpsum = octx.enter_context(
                tc.tile_pool(name="hpsum", bufs=2, space="PSUM")
            )
            NB = CHID // 512  # 4 zp column blocks
            for i in range(NAG):
                with contextlib.ExitStack() as ctx:
                    _matmul_custom(
                        ctx, tc, at8_c[:, :, :], p_f[i][:, :],
                        _m4_reducer(
                            nc, None if b2_zero else b2_sb, hm_parts, i * NTQ
                        ),
                        noop,
                        # relu output is scratch (only the fused row-sum is
                        # kept), so fp8 minimizes its SBUF footprint
                        output_type=FP8,
                        psum_n_bufs=1,
                        kxm_cache=at8_c[:, :, :],
                        # 512-k tiles: the first p_f DMA after each
                        # AllGather is 256KB instead of 1MB, shortening the
                        # post-collective restart latency
                        max_k_tile=512,
                        kxn_pool=m4_kxn_pool,
                    )
                if i == 0:
                    nc.sync.dma_start(
                        out=wc1_t[:, :, :],
                        in_=wc1[:, :].rearrange("(po pi) n -> pi po n", pi=128),
                    )
                    nc.sync.dma_start(out=bc1_t[:, :], in_=bc1[:, :])
                    nc.sync.dma_start(out=wc2_t[:, :], in_=wc2[:, :])
                # split-z: this chunk's hm contribution -> zp_i -> AllReduce,
                # so chunk 0's AR rides under chunk 1's aggregation and only
                # the last AR sits on the tail
                nc.vector.tensor_reduce(
                    out=hm_i[i][:, :],
                    in_=hm_parts[:, :, i * NTQ : (i + 1) * NTQ],
                    axis=mybir.AxisListType.X, op=mybir.AluOpType.add,
                )
                nc.vector.tensor_scalar_mul(
                    hm_i[i][:, :], hm_i[i][:, :],
                    1.0 / (HID * (WSCALE if USE_FP8 else 1.0)),
                )
                nc.vector.tensor_copy(out=hm_ib[i][:, :], in_=hm_i[i][:, :])
                for j in range(NB):
                    psj = hpsum.tile([128, 512], F32, name="zpps")
                    for ko in range(MSUB):
                        nc.tensor.matmul(
                            psj[0:1, :],
                            hm_ib[i][:, ko : ko + 1],
                            wc1_t[:, ko, 512 * j : 512 * (j + 1)],
                            start=(ko == 0),
                            stop=(ko == MSUB - 1),
                        )
                    nc.vector.tensor_copy(
                        out=zp_t[i][:, 512 * j : 512 * (j + 1)], in_=psj[0:1, :]
                    )
                nc.sync.dma_start(out=zb[i][:, :], in_=zp_t[i][:, :])
                nc.gpsimd.collective_compute(
                    "AllReduce",
                    mybir.AluOpType.add,
                    ins=[zb[i][:, :].opt()],
                    outs=[zf[i][:, :].opt()],
                    replica_groups=[list(range(NCORES))],
                )
            # epilogue on z viewed as [128, 16] so the DVE ops use all lanes
            psr = hpsum.tile([128, 512], F32, name="zpps")
            nc.sync.dma_start(
                out=z2_t[:, :], in_=zf[0][:, :].rearrange("o (p i) -> p (o i)", p=128)
            )
            nc.sync.dma_start(
                out=z2b_t[:, :], in_=zf[1][:, :].rearrange("o (p i) -> p (o i)", p=128)
            )
            nc.vector.tensor_add(out=z2_t[:, :], in0=z2_t[:, :], in1=z2b_t[:, :])
            nc.vector.tensor_add(out=z2_t[:, :], in0=z2_t[:, :], in1=bc1_t[:, :])
            nc.vector.tensor_scalar_max(z2_t[:, :], z2_t[:, :], 0.0)
            nc.vector.tensor_mul(out=z2_t[:, :], in0=z2_t[:, :], in1=wc2_t[:, :])
            nc.vector.tensor_reduce(
                out=zcol_t[:, :], in_=z2_t[:, :],
                axis=mybir.AxisListType.X, op=mybir.AluOpType.add,
            )
            # cross-partition sum via a 128x1 ones matmul
            nc.tensor.matmul(
                psr[0:1, 0:1], ones_t[:, 0:1], zcol_t[:, 0:1], start=True, stop=True
            )
            nc.vector.tensor_copy(out=r_t[:, :], in_=psr[0:1, 0:1])
            nc.sync.dma_start(out=res[:, :], in_=r_t[:, :])

    nc.compile()
    nc.m = get_hw_module(nc.m)
    return nc


def get_compiled(b1_zero=True, b2_zero=True):
    key = (b1_zero, b2_zero)
    if key not in _COMPILED:
        _COMPILED[key] = _build_graph(*key)
    return _COMPILED[key]


def _bf16(a):
    return np.ascontiguousarray(np.asarray(a, dtype=np.float32)).astype(ml_dtypes.bfloat16)


def _f32(a):
    return np.ascontiguousarray(np.asarray(a, dtype=np.float32))


_NP_FP8 = mybir.dt.np(FP8)


def _adt(a):
    """Convert to the aggregation dtype (fp8 or bf16)."""
    a = np.ascontiguousarray(np.asarray(a, dtype=np.float32))
    return a.astype(_NP_FP8 if USE_FP8 else ml_dtypes.bfloat16)


def make_in_maps(x, edge_index, W_embed, b_embed, W1, b1, W2, b2, Wc1, bc1, Wc2, bc2):
    x = _f32(x)
    ei = np.asarray(edge_index).astype(np.int64)
    # adjacency counts, transposed: AT[src, dst] = #edges src->dst
    counts = np.bincount(ei[1] * N + ei[0], minlength=N * N).astype(np.float32)
    AT = counts.reshape(N, N)

    # padded to KEP so M1 computes the tT DoubleRow-pad rows as real zeros
    x_ext = np.zeros((N, KEP), np.float32)
    x_ext[:, :IN_DIM] = x
    x_ext[:, IN_DIM] = 1.0

    we_ext = np.zeros((KEP, HID), np.float32)
    we_ext[:IN_DIM] = _f32(W_embed).T
    we_ext[IN_DIM] = _f32(b_embed)
    # layer-1 transform is low-rank: fold We_ext.T @ W1.T on the host
    wcb_full = we_ext @ _f32(W1).T
    if M3_FP8:
        # scale into e4m3's normal range; h1T then carries WCB_SCALE and
        # the p eviction divides it back out
        wcb_np = _adt(wcb_full * WCB_SCALE)
    else:
        wcb_np = _bf16(wcb_full)

    xe_np = _adt(x_ext)
    at8_np = _adt(AT)
    wmul = WSCALE if USE_FP8 else 1.0
    w2_np = _adt(_f32(W2).T * wmul) if USE_FP8 else _bf16(_f32(W2).T)
    # b1 per-partition layout for the feature-major h1T eviction (h1T
    # carries the WCB_SCALE factor, so b1 must too)
    b1s = _f32(b1) * (WCB_SCALE if M3_FP8 else 1.0)
    b1c_np = _f32(np.ascontiguousarray(b1s.reshape(HID // 128, 128).T))
    b2s = _f32(b2) * (WSCALE if USE_FP8 else 1.0)
    b2_np = _f32(np.broadcast_to(b2s, (128, HID)))
    wc1T = _bf16(_f32(Wc1).T)  # [HID(nodes), CHID] bf16
    wc2_row = _f32(Wc2).reshape(128, CHID // 128)
    bc1_full = _f32(bc1).reshape(128, CHID // 128)

    in_maps = []
    for c in range(NCORES):
        rows = slice(S * c, S * (c + 1))
        in_maps.append(
            {
                "xe": xe_np,
                "wcb": wcb_np,
                "at8": np.ascontiguousarray(at8_np[:, rows]),
                "w2": w2_np,
                "b1": b1c_np,
                "b2": b2_np,
                "wc1": np.ascontiguousarray(wc1T[rows, :]),
                "bc1": bc1_full,
                "wc2": wc2_row,
            }
        )
    return in_maps


def kernel(**inputs):
    b1_zero = not np.any(np.asarray(inputs["b1"], dtype=np.float32))
    b2_zero = not np.any(np.asarray(inputs["b2"], dtype=np.float32))
    nc = get_compiled(b1_zero, b2_zero)
    in_maps = make_in_maps(**inputs)
    bres = run_bass_kernel_spmd(nc, in_maps, core_ids=list(range(NCORES)))
    val = np.float32(bres.results[0]["res"][0, 0])
    bc2 = np.asarray(inputs["bc2"], dtype=np.float32).reshape(-1)
    out = np.asarray(val + bc2[0], dtype=np.float32).reshape(())
    return out


# revision 44
# speedup vs baseline: 1.0251x; 1.0085x over previous
"""Trainium2 Bass kernel for a 2-layer message-passing GNN (BaselineGNN).

Reference computation (N=4096 nodes, IN=512, HID=4096, E=65536 edges):
    h   = x @ We.T + be                                   [N, HID]
    for W, b in ((W1, b1), (W2, b2)):
        aggr = segment_sum(h[col], row)                   [N, HID]
        h    = relu(aggr @ W.T + b)
    hm  = mean(h, axis=1)                                 [N]
    z   = relu(hm @ Wc1.T + bc1)                          [HID//2]
    out = (z @ Wc2.T + bc2).squeeze(-1)                   scalar

Strategy (8 NeuronCores, node-parallel, collective-hiding reorder):
  * segment_sum == A @ h with A the [N, N] adjacency-count matrix (0.4%
    dense).  A's entries are small integer counts -> exactly representable
    in fp8-e4m3, so aggregation runs as a dense TensorEngine matmul.
  * Nodes are sharded: core c owns rows 512c..512c+512.
  * Layer 1 is low-rank through the embed bottleneck:
        h1_c = relu((A_c @ x_ext) @ (We_ext.T @ W1.T) + b1)
    with the weight product folded on the host (x_ext carries a ones
    column, We_ext.T a b_embed row).
  * Layer 2 is REORDERED vs the naive form to hide the collective:
        h2 = relu(A @ h1 @ W2.T + b2) = relu(A @ (h1 @ W2.T) + b2)
    Each core computes p_c = h1_c @ W2.T *locally* (W2 replicated) BEFORE
    any cross-core exchange -- ~150us of PE work that overlaps the
    first-collective rendezvous/launch-skew barrier AND the p AllGather.
    p is AllGathered in column halves; the final aggregation
    aggr2 = A_c @ p_full then runs per half with relu+b2+row-sum fused
    into the PSUM eviction (ScalarE accum_out) -> hm directly.  No
    second weight matmul after the collective, so the PE pipeline is
    near-continuous: M1 -> M3 -> M5 (local, hides AG) -> M4 -> head.
  * Pipeline per core: M1 tT=(A_c@x_ext).T [fp8 DR, split into source
    halves so it starts before the operand caches fully land] -> M3 h1T
    [bf16, feature-major so h1T is the kxm cache for M5] -> M5 halves
    p_c = h1T.T @ (W2.T*16) [fp8 DR] -> AllGather half (fp8) ->
    M4 halves aggr2 = at8.T @ p_full [fp8 DR] with relu+row-sum fused
    eviction -> per-half partial z = Wc1@hm_half AllReduced early so only
    the last AR sits on the tail -> epilogue.
  * Chained matmuls hand intermediates over in SBUF caches (tT_c, h1T_c,
    at8_c) via mxn_subtile_producer / kxm/kxn_cache.
  * fp8 operands that are not exact (x, h1, p, W2) rely on fp32 PSUM
    accumulation and the mean-pool/classifier averaging to wash the 6%
    element-level rounding down to ~4e-4 relative error on the scalar
    output; W2 is pre-scaled by 16 to sit in e4m3's normal range (the
    hm normalization divides it back out).  The classifier runs bf16/fp32.
  * A dummy AllReduce issued first absorbs the multi-core launch skew on
    the collective engine while the PE computes M1/M3/M5.

Everything is expressed "transposed-free": every matmul is
mxn = kxm.T @ kxn with operands stored so no transposes are ever needed.
"""

import contextlib

import numpy as np
import ml_dtypes

import concourse.bass as bass
import concourse.mybir as mybir
import concourse.tile as tile
from concourse import bacc
from concourse.bass_interp import get_hw_module
from concourse.bass_utils import run_bass_kernel_spmd
from concourse.kernels.tile_matmul import (
    composable_matmul_tile_kernel,
    dma_from_dram_kxm,
    dma_from_dram_kxn,
    dma_to_dram_mxn,
    k_pool_min_bufs,
    scalar_copyback,
    scalar_scale,
)

N = 4096          # nodes
IN_DIM = 512
HID = 4096
NCORES = 8
S = N // NCORES           # nodes per core (512)
KE = 640                  # extended embed contraction (512 + 1 ones col, padded to 5*128)
CHID = HID // 2           # classifier hidden (2048)

BF16 = mybir.dt.bfloat16
F32 = mybir.dt.float32
FP8 = mybir.dt.float8e4

USE_FP8 = True
# p = h1 @ (W2.T * WSCALE) is materialized in fp8-e4m3 (range +-448):
# p_true has absmax ~12, so WSCALE=16 centers it (~200 max) with margin.
WSCALE = 16.0
# layer-1 path in fp8 DoubleRow: wcb is scaled by WCB_SCALE (its entries
# are ~9e-3, below e4m3's normal range), h1T carries the factor (absmax
# 15.6*16=250 < 448; x32 would clip), and the p eviction divides it back
# out so p stays WSCALE-scaled.
M3_FP8 = False
WCB_SCALE = 16.0
KEP = 768 if M3_FP8 else KE   # KE padded to a DoubleRow-even k (6*128)

NAG = 2                   # AllGather column-chunk count
HQ = HID // NAG

_COMPILED = {}


def _m4_reducer(nc_b, bias_sb, accum, n_off):
    """PSUM->SBUF evict for the final aggregation:
    sbuf = relu(psum + b2[n_slice]); row-sums stream into accum.

    bias_sb: [128, HID] SBUF tile (b2 broadcast across partitions), or None
    when b2 is all-zero.  accum: [128, MSUB, NT] fp32; the relu
    row-sum-accumulates into accum[:, m_subtile, n_off + n_tile].
    """

    def _reducer(nc, psum, sbuf, md):
        src_ap = psum[:, : md.n_slice_size]
        ni = n_off + md.n_tile_idx * md.n_subtiles + md.n_subtile_idx
        if bias_sb is not None:
            start = (
                n_off * md.n_tile
                + md.n_tile_idx * md.n_tile
                + md.n_subtile_idx * md.n_subtile
            )
            nc.vector.tensor_add(
                out=sbuf[:, :, : md.n_slice_size],
                in0=src_ap,
                in1=bias_sb[:, start : start + md.n_slice_size],
            )
            src_ap = sbuf
        nc.scalar.activation(
            out=sbuf,
            in_=src_ap,
            func=mybir.ActivationFunctionType.Relu,
            accum_out=accum[:, md.m_subtile_idx, ni : ni + 1],
        )

    return _reducer


def _m1_combine_reducer(tT_h, tT_c):
    """Second M1 half eviction: tT_c = fp(psum + tT_h) (combine k-halves)."""

    def _reducer(nc, psum, sbuf, md):
        mi = md.m_subtiles * md.m_tile_idx + md.m_subtile_idx
        nc.vector.tensor_add(
            out=tT_c[:, mi : mi + 1, : md.n_slice_size],
            in0=psum[:, : md.n_slice_size],
            in1=tT_h[:, mi : mi + 1, : md.n_slice_size],
        )

    return _reducer


def _m3_reducer(nc_b, b1c_sb):
    """PSUM->SBUF evict for h1T: sbuf = relu(psum + b1[m_slice]).

    h1T is feature-major, so the layer-1 bias is per-partition:
    b1c_sb is [128, HID//128] with b1c[p, m] = b1[m*128 + p], or None
    when b1 is all-zero.
    """

    def _reducer(nc, psum, sbuf, md):
        if b1c_sb is None:
            nc.scalar.activation(
                out=sbuf, in_=psum, func=mybir.ActivationFunctionType.Relu
            )
        else:
            mi = md.m_tile_idx * md.m_subtiles + md.m_subtile_idx
            nc.vector.tensor_add(
                out=sbuf[:, :, : md.n_slice_size],
                in0=psum[:, : md.n_slice_size],
                in1=b1c_sb[:, mi : mi + 1].to_broadcast(
                    [128, 1, md.n_slice_size]
                ),
            )
            nc.vector.tensor_scalar_max(sbuf, sbuf, 0.0)

    return _reducer


def _matmul_custom(ctx, tc, kxm_ap, kxn_ap, reducer, consumer, output_type=F32,
                   psum_n_bufs=2, kxm_cache=None, kxn_cache=None, producer=None,
                   max_k_tile=512, stream_ap=None, kxn_pool=None,
                   reducer_override=None):
    """composable matmul with custom psum reducer / mxn consumer / SBUF caches.

    kxm_cache / kxn_cache: [128, K//128, M-or-N] SBUF tiles already holding
    the operand (no DMA is issued for that side).  producer: optional
    mxn_subtile_producer returning the SBUF tile the reducer writes.
    stream_ap: the DRAM-streamed operand used to size the DMA pools
    (defaults to kxn_ap).  kxn_pool: pre-created pool shared across calls
    so the next call's first tiles prefetch during the previous call.
    """
    nc = tc.nc
    if stream_ap is None:
        stream_ap = kxn_ap if kxn_cache is None else kxm_ap
    if kxm_cache is not None and kxn_cache is not None:
        num_bufs = 2
    else:
        num_bufs = k_pool_min_bufs(stream_ap, max_tile_size=max_k_tile)
    if kxm_cache is None:
        kxm_pool = ctx.enter_context(
            tc.tile_pool(name="kxm_pool", bufs=num_bufs + 4)
        )
    else:
        kxm_pool = None
    if kxn_cache is None:
        if kxn_pool is None:
            kxn_pool = ctx.enter_context(tc.tile_pool(name="kxn_pool", bufs=num_bufs))
    else:
        kxn_pool = None
    kxm_producer, kxm_shape = dma_from_dram_kxm(kxm_pool, kxm_ap, kxm_cache=kxm_cache)
    kxn_producer, kxn_shape = dma_from_dram_kxn(kxn_pool, kxn_ap, kxn_cache=kxn_cache)
    composable_matmul_tile_kernel(
        tc=tc,
        kxm_shape=kxm_shape,
        kxn_shape=kxn_shape,
        output_type=output_type if producer is None else None,
        kxm_producer=kxm_producer,
        kxn_producer=kxn_producer,
        mxn_subtile_reducer=reducer if reducer_override is None else reducer_override,
        mxn_consumer=consumer,
        mxn_subtile_producer=producer,
        psum_n_bufs=psum_n_bufs,
        MAX_K_TILE_SIZE=max_k_tile,
    )


def _build_graph(b1_zero=True, b2_zero=True):
    nc = bacc.Bacc(
        "TRN2",
        target_bir_lowering=False,
        debug=False,
        enable_asserts=False,
        num_devices=NCORES,
    )

    ADT = FP8 if USE_FP8 else BF16

    # ---- kernel I/O (per core) ----
    WBT = ADT if M3_FP8 else BF16
    xe = nc.dram_tensor("xe", [N, KEP], ADT, kind="ExternalInput")        # x_ext (replicated, padded)
    at8 = nc.dram_tensor("at8", [N, S], ADT, kind="ExternalInput")        # A.T[:, rows_c] (sharded)
    wcb = nc.dram_tensor("wcb", [KEP, HID], WBT, kind="ExternalInput")    # We_ext.T @ W1.T (replicated)
    w2 = nc.dram_tensor("w2", [HID, HID], ADT, kind="ExternalInput")      # W2.T * WSCALE (replicated)
    b1 = nc.dram_tensor("b1", [128, HID // 128], F32, kind="ExternalInput")  # b1 partition-major
    b2 = nc.dram_tensor("b2", [128, HID], F32, kind="ExternalInput")      # b2*WSCALE bcast (replicated)
    wc1 = nc.dram_tensor("wc1", [S, CHID], BF16, kind="ExternalInput")    # Wc1.T row-chunk (sharded)
    bc1 = nc.dram_tensor("bc1", [128, CHID // 128], F32, kind="ExternalInput")  # bc1 [128,16]
    wc2 = nc.dram_tensor("wc2", [128, CHID // 128], F32, kind="ExternalInput")  # Wc2 [128,16]
    res = nc.dram_tensor("res", [1, 1], F32, kind="ExternalOutput")       # final scalar (pre-bc2)

    # ---- internal DRAM ----
    # p = h1 @ (W2.T*64) is exchanged in column halves so each AllGather
    # overlaps with the production of the next half (M5) / consumption of
    # the previous one (M4).
    p_c = [nc.dram_tensor(f"pc{i}", [S, HQ], ADT) for i in range(NAG)]
    p_f = [
        nc.dram_tensor(f"pf{i}", [N, HQ], ADT, addr_space="Shared")
        for i in range(NAG)
    ]
    zb = [nc.dram_tensor(f"zb{i}", [1, CHID], F32) for i in range(NAG)]
    zf = [
        nc.dram_tensor(f"zf{i}", [1, CHID], F32, addr_space="Shared")
        for i in range(NAG)
    ]
    da = nc.dram_tensor("da", [1, 8], F32)              # launch-skew sync dummy
    df = nc.dram_tensor("df", [1, 8], F32, addr_space="Shared")

    MSUB = S // 128   # 4 m-subtiles in a 512-row tile
    NT = HID // 512   # 8 n-tiles of 512
    NTQ = NT // NAG   # n-tiles per AG half

    with tile.TileContext(nc) as tc:
        with contextlib.ExitStack() as octx:
            const = octx.enter_context(tc.tile_pool(name="const", bufs=1))
            b2_sb = (
                None if b2_zero else const.tile([128, HID], F32, name="b2_sb")
            )
            b1c_sb = const.tile([128, HID // 128], F32, name="b1c_sb")
            hm_parts = const.tile([128, MSUB, NT], F32, name="hm_parts")
            hm_sb = const.tile([128, MSUB], F32, name="hm_sb")
            hm_bf = const.tile([128, MSUB], BF16, name="hm_bf")
            nc.any.memset(hm_parts[:], 0.0)

            head = octx.enter_context(tc.tile_pool(name="head", bufs=1))
            CI = CHID // 128  # 16
            wc1_t = head.tile([128, MSUB, CHID], BF16, name="wc1_t")
            bc1_t = head.tile([128, CI], F32, name="bc1_t")
            wc2_t = head.tile([128, CI], F32, name="wc2_t")
            zp_t = [head.tile([1, CHID], F32, name=f"zp_t{i}") for i in range(NAG)]
            hm_i = [head.tile([128, MSUB], F32, name=f"hm_i{i}") for i in range(NAG)]
            hm_ib = [head.tile([128, MSUB], BF16, name=f"hm_ib{i}") for i in range(NAG)]
            z2_t = head.tile([128, CI], F32, name="z2_t")
            z2b_t = head.tile([128, CI], F32, name="z2b_t")
            zcol_t = head.tile([128, 1], F32, name="zcol_t")
            ones_t = head.tile([128, 1], F32, name="ones_t")
            r_t = head.tile([1, 1], F32, name="r_t")
            nc.any.memset(ones_t[:], 1.0)

            # persistent SBUF caches for chained-matmul intermediates
            cache = octx.enter_context(tc.tile_pool(name="cache", bufs=1))
            at8_c = cache.tile([128, N // 128, S], ADT, name="at8_c")
            # xe / tT_h live only until M1 completes; their pool closes
            # right after so the M5/M4 stream pools can reuse the space
            m1ctx = contextlib.ExitStack()
            m1pool = m1ctx.enter_context(tc.tile_pool(name="m1pool", bufs=1))
            xe_h = [
                m1pool.tile([128, N // 256, KEP], ADT, name=f"xe_h{i}")
                for i in range(2)
            ]
            at8_h = [
                m1pool.tile([128, N // 256, S], ADT, name=f"at8_h{i}")
                for i in range(2)
            ]
            tT_c = cache.tile([128, KEP // 128, S], WBT, name="tT_c")
            tT_h = m1pool.tile([128, KEP // 128, S], BF16, name="tT_h")
            h1T_c = cache.tile([128, HID // 128, S], ADT, name="h1T_c")
            # both M1 operands are fully cached via big contiguous DMAs
            # (streaming xe per-tile emits 128B descriptor rows -> ~5x slow);
            # chunked + interleaved so M1's first k-tile starts early
            wcb_c = cache.tile([128, KEP // 128, HID], WBT, name="wcb_c")
            at8_r = at8[:, :].rearrange("(po pi) n -> pi po n", pi=128)
            xe_r = xe[:, :].rearrange("(po pi) n -> pi po n", pi=128)
            wcb_r = wcb[:, :].rearrange("(po pi) n -> pi po n", pi=128)
            HK = N // 256  # k-subtiles per xe half (16)
            # M1's operands go to half-tiles so its first matmuls depend on
            # only ~2.3MB of DMA (dep tracking is tile-granular); M4's full
            # at8_c cache is re-read from DRAM later, hidden under M3/M5
            for po in range(0, N // 256, 4):
                nc.sync.dma_start(out=at8_h[0][:, po : po + 4, :], in_=at8_r[:, po : po + 4, :])
                nc.sync.dma_start(out=xe_h[0][:, po : po + 4, :], in_=xe_r[:, po : po + 4, :])
            for po in range(N // 256, N // 128, 4):
                nc.sync.dma_start(
                    out=at8_h[1][:, po - HK : po - HK + 4, :],
                    in_=at8_r[:, po : po + 4, :],
                )
                nc.sync.dma_start(
                    out=xe_h[1][:, po - HK : po - HK + 4, :],
                    in_=xe_r[:, po : po + 4, :],
                )
            # wcb cached too (M3's weight side): issued after the M1 operands
            # so it lands just as M1 finishes
            for po in range(KEP // 128):
                nc.sync.dma_start(
                    out=wcb_c[:, po : po + 1, :], in_=wcb_r[:, po : po + 1, :]
                )
            noop = lambda nc_, sbuf, md: None

            # dummy AllReduce fired first: absorbs the multi-core launch skew
            # on the collective engine while the PE is busy with M1/M3/M5, so
            # the p AllGathers later only see compute drift
            sync_t = head.tile([1, 8], F32, name="sync_t")
            nc.any.memset(sync_t[:], 0.0)
            nc.sync.dma_start(out=da[:, :], in_=sync_t[:, :])
            nc.gpsimd.collective_compute(
                "AllReduce",
                mybir.AluOpType.add,
                ins=[da[:, :].opt()],
                outs=[df[:, :].opt()],
                replica_groups=[list(range(NCORES))],
            )

            # M1: tT = (A_c @ x_ext).T = xe.T @ A_c.T        [KE, S]
            # run as two source-half matmuls so the first starts when only
            # half the operand caches have landed (tile-granular DMA deps);
            # halves are combined with a cheap DVE add into tT_c
            for hi in range(2):
                with contextlib.ExitStack() as ctx:
                    _matmul_custom(
                        ctx, tc, xe_h[hi][:, :, :],
                        at8_h[hi][:, :, :],
                        scalar_copyback(), noop,
                        kxm_cache=xe_h[hi][:, :, :],
                        kxn_cache=at8_h[hi][:, :, :],
                        producer=(
                            (lambda nc_, md: tT_h[
                                :,
                                md.m_subtiles * md.m_tile_idx : md.m_subtiles
                                * (md.m_tile_idx + 1),
                                :,
                            ])
                            if hi == 0
                            else None
                        ),
                        output_type=None if hi == 0 else F32,
                        reducer_override=(
                            None
                            if hi == 0
                            else _m1_combine_reducer(tT_h, tT_c)
                        ),
                        max_k_tile=512,
                    )
            m1ctx.close()
            # b1 prefetch after M1 so it doesn't starve M1's tiles in the
            # DMA queues (scheduler priority = trace order)
            if not b1_zero:
                nc.sync.dma_start(out=b1c_sb[:, :], in_=b1[:, :])

            # M3: h1T = relu((We_ext.T W1.T).T @ t.T + b1)   [HID, S]
            # feature-major so h1T is directly the kxm cache for M5
            with contextlib.ExitStack() as ctx:
                _matmul_custom(
                    ctx, tc, wcb_c[:, :, :], tT_c[:, :, :],
                    _m3_reducer(nc, None if b1_zero else b1c_sb),
                    noop,
                    kxm_cache=wcb_c[:, :, :],
                    kxn_cache=tT_c[:, :, :],
                    producer=lambda nc_, md: h1T_c[
                        :, MSUB * md.m_tile_idx : MSUB * (md.m_tile_idx + 1), :
                    ],
                    # 256 keeps K_SUBTILES even so fp8 DoubleRow engages
                    max_k_tile=256 if M3_FP8 else 512,
                )

            # M4's at8 cache loads here, after M3's operands, so it fills
            # DMA idle slots under M3/M5 without delaying wcb
            for po in range(0, N // 128, 8):
                nc.sync.dma_start(
                    out=at8_c[:, po : po + 8, :], in_=at8_r[:, po : po + 8, :]
                )

            # M5 halves: p[:, half] = h1T.T @ w2[:, half]    [S, HQ] fp8
            # entirely local (W2 replicated) -- this is the PE work that
            # hides the collective rendezvous + AllGather; each half's AG
            # fires as soon as the half is in DRAM.  With M3_FP8 both h1T
            # and w2 carry a WSCALE factor, so the eviction divides one out.
            m5_evict = (
                scalar_scale(1.0 / WCB_SCALE) if M3_FP8 else scalar_copyback()
            )
            m5_kxn_pool = octx.enter_context(
                tc.tile_pool(name="m5_kxn_pool", bufs=7)
            )
            for i in range(NAG):
                cons = dma_to_dram_mxn(p_c[i][:, :])
                with contextlib.ExitStack() as ctx:
                    _matmul_custom(
                        ctx, tc, h1T_c[:, :, :], w2[:, i * HQ : (i + 1) * HQ],
                        m5_evict, cons,
                        output_type=ADT,
                        kxm_cache=h1T_c[:, :, :],
                        max_k_tile=2048,
                        kxn_pool=m5_kxn_pool,
                    )
                nc.gpsimd.collective_compute(
                    "AllGather",
                    mybir.AluOpType.bypass,
                    ins=[p_c[i][:, :].opt()],
                    outs=[p_f[i][:, :].opt()],
                    replica_groups=[list(range(NCORES))],
                )
            # b2 prefetch (needed by M4's reducer) after the AG triggers so
            # the p bounce writes aren't queued behind it
            if not b2_zero:
                nc.sync.dma_start(out=b2_sb[:, :], in_=b2[:, :])

            # M4 chunks: aggr2[:, chunk] = at8.T @ p_full[:, chunk]  [S, HQ]
            # relu+b2 fused into the eviction; row-sums stream into hm_parts.
            # w2 was pre-scaled by WSCALE and b2 holds WSCALE*b2, so the
            # accumulated sums are WSCALE*h2; the hm normalization divides
            # it back out.
            m4_kxn_pool = octx.enter_context(
                tc.tile_pool(name="m4_kxn_pool", bufs=7)
            )
            # head psum pool coexists with M4's (psum_n_bufs=1 there):
            # 4 + 3 banks <= 8
            hpsum = octx.enter_context(
                tc.tile_pool(name="hpsum", bufs=2, space="PSUM")
            )
            NB = CHID // 512  # 4 zp column blocks
            for i in range(NAG):
                with contextlib.ExitStack() as ctx:
                    _matmul_custom(
                        ctx, tc, at8_c[:, :, :], p_f[i][:, :],
                        _m4_reducer(
                            nc, None if b2_zero else b2_sb, hm_parts, i * NTQ
                        ),
                        noop,
                        # relu output is scratch (only the fused row-sum is
                        # kept), so fp8 minimizes its SBUF footprint
                        output_type=FP8,
                        psum_n_bufs=1,
                        kxm_cache=at8_c[:, :, :],
                        # 512-k tiles: the first p_f DMA after each
                        # AllGather is 256KB instead of 1MB, shortening the
                        # post-collective restart latency
                        max_k_tile=512,
                        kxn_pool=m4_kxn_pool,
                    )
                if i == 0:
                    nc.sync.dma_start(
                        out=wc1_t[:, :, :],
                        in_=wc1[:, :].rearrange("(po pi) n -> pi po n", pi=128),
                    )
                    nc.sync.dma_start(out=bc1_t[:, :], in_=bc1[:, :])
                    nc.sync.dma_start(out=wc2_t[:, :], in_=wc2[:, :])
                # split-z: this chunk's hm contribution -> zp_i -> AllReduce,
                # so chunk 0's AR rides under chunk 1's aggregation and only
                # the last AR sits on the tail
                nc.vector.tensor_reduce(
                    out=hm_i[i][:, :],
                    in_=hm_parts[:, :, i * NTQ : (i + 1) * NTQ],
                    axis=mybir.AxisListType.X, op=mybir.AluOpType.add,
                )
                nc.vector.tensor_scalar_mul(
                    hm_i[i][:, :], hm_i[i][:, :],
                    1.0 / (HID * (WSCALE if USE_FP8 else 1.0)),
                )
                nc.vector.tensor_copy(out=hm_ib[i][:, :], in_=hm_i[i][:, :])
                for j in range(NB):
                    psj = hpsum.tile([128, 512], F32, name="zpps")
                    for ko in range(MSUB):
                        nc.tensor.matmul(
                            psj[0:1, :],
                            hm_ib[i][:, ko : ko + 1],
                            wc1_t[:, ko, 512 * j : 512 * (j + 1)],
                            start=(ko == 0),
                            stop=(ko == MSUB - 1),
                        )
                    nc.vector.tensor_copy(
                        out=zp_t[i][:, 512 * j : 512 * (j + 1)], in_=psj[0:1, :]
                    )
                nc.sync.dma_start(out=zb[i][:, :], in_=zp_t[i][:, :])
                nc.gpsimd.collective_compute(
                    "AllReduce",
                    mybir.AluOpType.add,
                    ins=[zb[i][:, :].opt()],
                    outs=[zf[i][:, :].opt()],
                    replica_groups=[list(range(NCORES))],
                )
            # epilogue on z viewed as [128, 16] so the DVE ops use all lanes
            psr = hpsum.tile([128, 512], F32, name="zpps")
            nc.sync.dma_start(
                out=z2_t[:, :], in_=zf[0][:, :].rearrange("o (p i) -> p (o i)", p=128)
            )
            nc.sync.dma_start(
                out=z2b_t[:, :], in_=zf[1][:, :].rearrange("o (p i) -> p (o i)", p=128)
            )
            nc.vector.tensor_add(out=z2_t[:, :], in0=z2_t[:, :], in1=z2b_t[:, :])
            nc.vector.tensor_add(out=z2_t[:, :], in0=z2_t[:, :], in1=bc1_t[:, :])
            nc.vector.tensor_scalar_max(z2_t[:, :], z2_t[:, :], 0.0)
            nc.vector.tensor_mul(out=z2_t[:, :], in0=z2_t[:, :], in1=wc2_t[:, :])
            nc.vector.tensor_reduce(
                out=zcol_t[:, :], in_=z2_t[:, :],
                axis=mybir.AxisListType.X, op=mybir.AluOpType.add,
            )
            # cross-partition sum via a 128x1 ones matmul
            nc.tensor.matmul(
                psr[0:1, 0:1], ones_t[:, 0:1], zcol_t[:, 0:1], start=True, stop=True
            )
            nc.vector.tensor_copy(out=r_t[:, :], in_=psr[0:1, 0:1])
            nc.sync.dma_start(out=res[:, :], in_=r_t[:, :])

    nc.compile()
    nc.m = get_hw_module(nc.m)
    return nc


def get_compiled(b1_zero=True, b2_zero=True):
    key = (b1_zero, b2_zero)
    if key not in _COMPILED:
        _COMPILED[key] = _build_graph(*key)
    return _COMPILED[key]


def _bf16(a):
    return np.ascontiguousarray(np.asarray(a, dtype=np.float32)).astype(ml_dtypes.bfloat16)


def _f32(a):
    return np.ascontiguousarray(np.asarray(a, dtype=np.float32))


_NP_FP8 = mybir.dt.np(FP8)


def _adt(a):
    """Convert to the aggregation dtype (fp8 or bf16)."""
    a = np.ascontiguousarray(np.asarray(a, dtype=np.float32))
    return a.astype(_NP_FP8 if USE_FP8 else ml_dtypes.bfloat16)


def make_in_maps(x, edge_index, W_embed, b_embed, W1, b1, W2, b2, Wc1, bc1, Wc2, bc2):
    x = _f32(x)
    ei = np.asarray(edge_index).astype(np.int64)
    # adjacency counts, transposed: AT[src, dst] = #edges src->dst
    counts = np.bincount(ei[1] * N + ei[0], minlength=N * N).astype(np.float32)
    AT = counts.reshape(N, N)

    # padded to KEP so M1 computes the tT DoubleRow-pad rows as real zeros
    x_ext = np.zeros((N, KEP), np.float32)
    x_ext[:, :IN_DIM] = x
    x_ext[:, IN_DIM] = 1.0

    we_ext = np.zeros((KEP, HID), np.float32)
    we_ext[:IN_DIM] = _f32(W_embed).T
    we_ext[IN_DIM] = _f32(b_embed)
    # layer-1 transform is low-rank: fold We_ext.T @ W1.T on the host
    wcb_full = we_ext @ _f32(W1).T
    if M3_FP8:
        # scale into e4m3's normal range; h1T then carries WCB_SCALE and
        # the p eviction divides it back out
        wcb_np = _adt(wcb_full * WCB_SCALE)
    else:
        wcb_np = _bf16(wcb_full)

    xe_np = _adt(x_ext)
    at8_np = _adt(AT)
    wmul = WSCALE if USE_FP8 else 1.0
    w2_np = _adt(_f32(W2).T * wmul) if USE_FP8 else _bf16(_f32(W2).T)
    # b1 per-partition layout for the feature-major h1T eviction (h1T
    # carries the WCB_SCALE factor, so b1 must too)
    b1s = _f32(b1) * (WCB_SCALE if M3_FP8 else 1.0)
    b1c_np = _f32(np.ascontiguousarray(b1s.reshape(HID // 128, 128).T))
    b2s = _f32(b2) * (WSCALE if USE_FP8 else 1.0)
    b2_np = _f32(np.broadcast_to(b2s, (128, HID)))
    wc1T = _bf16(_f32(Wc1).T)  # [HID(nodes), CHID] bf16
    wc2_row = _f32(Wc2).reshape(128, CHID // 128)
    bc1_full = _f32(bc1).reshape(128, CHID // 128)

    in_maps = []
    for c in range(NCORES):
        rows = slice(S * c, S * (c + 1))
        in_maps.append(
            {
                "xe": xe_np,
                "wcb": wcb_np,
                "at8": np.ascontiguousarray(at8_np[:, rows]),
                "w2": w2_np,
                "b1": b1c_np,
                "b2": b2_np,
                "wc1": np.ascontiguousarray(wc1T[rows, :]),
                "bc1": bc1_full,
                "wc2": wc2_row,
            }
        )
    return in_maps


def kernel(**inputs):
    b1_zero = not np.any(np.asarray(inputs["b1"], dtype=np.float32))
    b2_zero = not np.any(np.asarray(inputs["b2"], dtype=np.float32))
    nc = get_compiled(b1_zero, b2_zero)
    in_maps = make_in_maps(**inputs)
    bres = run_bass_kernel_spmd(nc, in_maps, core_ids=list(range(NCORES)))
    val = np.float32(bres.results[0]["res"][0, 0])
    bc2 = np.asarray(inputs["bc2"], dtype=np.float32).reshape(-1)
    out = np.asarray(val + bc2[0], dtype=np.float32).reshape(())
    return out
